# revision 30
# baseline (speedup 1.0000x reference)
"""Trainium2 Bass kernel for nn_Chambers (6-tower MLP + coupled sigmoid recurrence).

Data-parallel over 8 NeuronCores: each core processes a 16384-sample shard in
16 chunks of 1024 samples.

- bf16 matmul pipeline: res is converted fp32->bf16 on DVE, PE-transposed in
  bf16 (1.0 cycles/row vs 2.0 for fp32), and L1-L4 run as bf16 matmuls with
  chambers packed block-diagonally. All h-activations are bf16 in SBUF.
- L3 chamber pair (4,5) is sample-folded into a [128, 512] tile so its SiLU
  costs 512 ACT rows instead of 1024.
- L4 accumulates raw directly into a persistent [96, 1024] PSUM tile across
  all 16 chunks via per-chunk W4 column stacks (chunk i writes rows 6i:6i+5;
  other rows accumulate zeros): no per-chunk raw bias-copy on ACT, no
  assembly DMAs.
- PSUM: 3 rotating [128,1024] matmul tags (6 banks) + the persistent raw
  accumulator (2 banks). Transpose tiles ride the tag rotation (bf16, half a
  slot).
- The 5-step coupled sigmoid recurrence runs fully in bf16 on quarter-width
  [96,256] chains with ping-pong act buffers, fed by a block-diagonal [96,96]
  matmul; the final iteration writes fp32.

Sync discipline (walrus allows 1 sem wait per instruction; Tile's dedup
clock advances only on real reads):
- psum tag pre-touches are 1x2 matmuls reading the tag's previous consumer
  and writing into the NEW tile itself (same-tile writes need no sem).
- one tiny ACT touch per chunk reads the previous chunk's h3a, advancing
  ACT's engine clock past every SiLU output-buffer WAW.
- DMA lanes are pre-observed by 1-wait touch reads (PE: wi/wa/wz into pm4
  scratch cells before its group opens; ACT/DVE: wf/res cells into SBUF
  scratch).
"""
import numpy as np
import ml_dtypes

import concourse.bass as bass
import concourse.mybir as mybir
from concourse.bass_utils import run_bass_kernel_spmd
from concourse.tile import TileContext
from concourse.tile_scheduler import N_PROCS
from concourse.vector_clock import ScopedClock
from bass_rust import add_dep_helper

F32 = mybir.dt.float32
BF16 = mybir.dt.bfloat16
AF = mybir.ActivationFunctionType
ALU = mybir.AluOpType

B = 131072
NCORES = 8
BS = B // NCORES           # 16384 samples per core
T = 1024                   # chunk (samples)
NCH = BS // T              # 16 chunks
RES_DIM = 100
CF_ITERS = 5
CF_K = 0.02

# wf (fp32) column layout: per-partition bias vectors
B1C = 0        # 6 cols (b1 per chamber, 128 rows)
B2PC = 6       # 3 cols (pair-packed b2: rows 0:64=b2[2pr], 64:128=b2[2pr+1])
B3PC = 9       # 1 col (b3 chambers 0-3 by 32s)
B3P2C = 10     # 1 col (b3 ch4,5 folded twice over 128 rows)
B4C = 11       # 1 col (b4 tiled x16 over 96 rows)
FCOLS = 12

# wi (bf16): identity + L1 weights (first DMA, gates compute start)
IDC = 0                    # identity [128,128] for PE transpose
W1C = 128                  # 6*128
ICOLS = W1C + 6 * 128

# wa (bf16): L2/L3 weights. All group-opening (start=True) matmuls use
# full-width [128,128] stationaries (zero-padded) so every psum partition
# is written: partial-width starts would leave unwritten rows accumulating
# onto stale bank data on hardware.
W2C = 0                    # 6*64
W2BC = W2C + 6 * 64        # 3*128: odd-chamber W2 shifted to out rows 64:127
W3AC = W2BC + 3 * 128      # 128: pair 0 in cols 0:64, zeros elsewhere
W3BC = W3AC + 128          # 128: pair 1 shifted to out rows 64:127
W3C1 = W3BC + 128          # 128: pair 2 in cols 0:64 (sample half 0)
W3C2 = W3C1 + 128          # 128: pair 2 in cols 64:128 (sample half 1)
ACOLS = W3C2 + 128

# wz (bf16): per-chunk W4 stacks + recurrence matrices
W4AC = 0                   # 16*96 (chambers 0-3)
W4BC = 16 * 96             # 16*96 (ch 4,5; both sample halves)
CDC = 2 * 16 * 96          # 96 (block-diag decay*coupling*k)
I96C = CDC + 96            # 96 (identity, for raw+delta accumulate)
ZCOLS = I96C + 96


class TC(TileContext):
    """TileContext with a walrus-compatible epilogue (split final waits)."""

    def _drain_and_barrier(self, tick_clock, wait_clock):
        nc = self.nc
        full = ScopedClock({None: tick_clock.global_clock})
        for scope, vc in full.items():
            for proc in range(N_PROCS):
                t = vc.peek_next(proc) - 1
                if t > 0:
                    sc = ScopedClock()
                    sc.require_at_least(scope, proc, t)
                    w = nc.sync.nop(nofuse=True)
                    wait_clock.add_sem_waits(w.ins, sc)
        for eng in nc.engines.values():
            eng.drain(fusable=False)
        nc.all_engine_barrier(sem_only=True)
        assert self.sems is not None
        popped = nc._tile_sem_poison_stack.pop()
        assert popped is self._sem_poison
        nc.clear_and_free_semaphores(list(self.sems.allocated().values()))
        for eng in nc.engines.values():
            eng.drain(fusable=False)
        nc.all_engine_barrier(sem_only=True)


def _order(after_inst, before_inst):
    if after_inst is not None and before_inst is not None:
        add_dep_helper(after_inst.ins, before_inst.ins, sync=False, reason="order")


def build_module():
    nc = bass.Bass()
    res_d = nc.dram_tensor("res", [BS, RES_DIM], F32, kind="ExternalInput")
    wf_d = nc.dram_tensor("wf", [128, FCOLS], F32, kind="ExternalInput")
    wi_d = nc.dram_tensor("wi", [128, ICOLS], BF16, kind="ExternalInput")
    wa_d = nc.dram_tensor("wa", [128, ACOLS], BF16, kind="ExternalInput")
    wz_d = nc.dram_tensor("wz", [128, ZCOLS], BF16, kind="ExternalInput")
    raw_d = nc.dram_tensor("raw_out", [96, T], F32, kind="ExternalOutput")
    act_d = nc.dram_tensor("act_out", [96, T], F32, kind="ExternalOutput")

    MMB = 3  # rotating matmul psum tags

    with TC(nc) as tc:
        with (
            tc.tile_pool(name="wconst", bufs=1) as wpool,
            tc.tile_pool(name="sbresb", bufs=1) as sbresb,
            tc.tile_pool(name="sbrt", bufs=1) as sbrt,
            tc.tile_pool(name="sbh", bufs=1) as sbh,
            tc.tile_pool(name="sbrec", bufs=1) as sbrec,
            tc.tile_pool(name="psmm", bufs=1, space="PSUM") as psmm,
            tc.tile_pool(name="psl4", bufs=1, space="PSUM") as psl4,
        ):
            # DMA issue order: transpose identity + W1 + chunk-0 res first so
            # compute starts quickly; L2/L3 weights next; W4 stacks and later
            # res chunks stream behind.
            wi = wpool.tile([128, ICOLS], BF16)
            nc.sync.dma_start(out=wi[:], in_=wi_d[:])
            res_sb0 = wpool.tile([128, 8 * RES_DIM], F32)
            nc.sync.dma_start(
                out=res_sb0[:],
                in_=res_d[0:T].rearrange("(p n) d -> p (n d)", p=128))
            wf = wpool.tile([128, FCOLS], F32)
            nc.sync.dma_start(out=wf[:], in_=wf_d[:])
            wa = wpool.tile([128, ACOLS], BF16)
            nc.sync.dma_start(out=wa[:], in_=wa_d[:])
            res_sb1 = wpool.tile([128, 3 * 8 * RES_DIM], F32)
            nc.sync.dma_start(
                out=res_sb1[:],
                in_=res_d[T:4 * T].rearrange("(p n) d -> p (n d)", p=128))
            wz = wpool.tile([128, ZCOLS], BF16)
            nc.sync.dma_start(out=wz[:], in_=wz_d[:])
            res_sb = wpool.tile([128, (NCH - 4) * 8 * RES_DIM], F32)
            nc.sync.dma_start(
                out=res_sb[:],
                in_=res_d[4 * T:].rearrange("(p n) d -> p (n d)", p=128))
            ident = wi[:, IDC:IDC + 128]

            raw_sb = sbrec.tile([96, T], F32)
            raw_r = sbrec.tile([96, T], BF16)
            act_r = sbrec.tile([96, T], BF16)
            act_r2 = sbrec.tile([96, T], BF16)
            act_o = sbrec.tile([96, T], F32)
            scr = sbrec.tile([1, 4], F32)
            scrA = sbrec.tile([1, 64], F32)
            scrD = sbrec.tile([1, 16], F32)

            pm4 = psl4.tile([96, T], F32)   # persistent raw accumulator

            pe_tail = None
            act_tail = None
            dve_tail = None
            acol = [0]
            dcol = [0]

            def pe_touch(src_ap, dst_ap):
                """1x2 matmul on PE reading src (observing its producer's
                sem lane) and writing scratch cells at dst (PSUM, f32)."""
                nonlocal pe_tail
                m = nc.tensor.matmul(dst_ap, src_ap[:, 0:1], src_ap[:, 0:2],
                                     start=True, stop=True)
                _order(m, pe_tail)
                pe_tail = m
                return m

            def act_touch(src_ap):
                """1-elem ACT copy reading src: advances ACT's observed
                clock past src's producer (absorbing later WAW waits)."""
                nonlocal act_tail
                c = acol[0]; acol[0] += 1
                assert c < 64
                s = nc.scalar.activation(scrA[0:1, c:c + 1], src_ap, AF.Copy)
                _order(s, act_tail)
                act_tail = s
                return s

            def dve_touch(src_ap):
                nonlocal dve_tail
                c = dcol[0]; dcol[0] += 1
                assert c < 16
                op = nc.vector.tensor_copy(scrD[0:1, c:c + 1], src_ap)
                _order(op, dve_tail)
                dve_tail = op
                return op

            tag_rr = [0]
            tag_state = [None] * MMB

            def new_mm_tile(name, width=T, dtype=F32, parts=128, touch=True):
                """Allocate the next rotating psum tag tile. For f32 tiles,
                pre-observe the tag's previous consumer with a touch matmul
                writing into the tile itself (same-tile writes carry no WAW
                sem; the real matmuls re-zero via start=True)."""
                tg = tag_rr[0] % MMB
                tag_rr[0] += 1
                t = psmm.tile([parts, width], dtype, tag=f"mm{tg}", bufs=1,
                              name=name)
                st = tag_state[tg]
                tag_state[tg] = None
                if st is not None and touch:
                    assert dtype == F32
                    tile_, row_, col_ = st
                    pe_touch(tile_[row_:row_ + 1, col_:col_ + 2], t[0:1, 0:2])
                return t, tg

            def mm(out_ap, lhs_ap, rhs_ap, **kw):
                nonlocal pe_tail
                m = nc.tensor.matmul(out_ap, lhs_ap, rhs_ap, **kw)
                _order(m, pe_tail)
                pe_tail = m
                return m

            def silu(out_ap, pm_ap, bias_ap, out_tile, tg, row, col,
                     func=AF.Silu):
                nonlocal act_tail
                s = nc.scalar.activation(out_ap, pm_ap, func, bias=bias_ap)
                _order(s, act_tail)
                act_tail = s
                if tg is not None:
                    tag_state[tg] = (out_tile, row, col)
                return s

            def dve(op):
                nonlocal dve_tail
                _order(op, dve_tail)
                dve_tail = op
                return op

            # Startup observes: PE on the wi lane (into a throwaway first
            # tag tile), ACT + DVE on the wf lane.
            tw0, _ = new_mm_tile("warm", width=2, parts=1, touch=False)
            pe_touch(wi[0:1, 0:2], tw0[0:1, 0:2])
            act_touch(wf[0:1, B1C:B1C + 1])
            dve_touch(wf[0:1, B1C:B1C + 1])

            def res_src(i):
                if i == 0:
                    return res_sb0, 0
                if i < 4:
                    return res_sb1, (i - 1) * 8 * RES_DIM
                return res_sb, (i - 4) * 8 * RES_DIM

            def emit_conv(i):
                """DVE: convert chunk i's res slice fp32 -> bf16."""
                if i == 4:
                    # observe the bulk-res DMA lane before conv(4) so it
                    # carries only its buffer WAW
                    dve_touch(res_sb[0:1, 0:1])
                rq, coff = res_src(i)
                rb = sbresb.tile([128, 8 * RES_DIM], BF16, tag="rb", bufs=2,
                                 name="rb")
                dve(nc.vector.tensor_copy(
                    rb[:], rq[:, coff:coff + 8 * RES_DIM]))
                return rb

            def emit_tr(i, rb):
                """PE transposes (into a rotating tag slot) + DVE copy ->
                rT [100, 1024] bf16 in SBUF. No pre-touch: callers order the
                allocation so the tag's WAR is already observed."""
                nonlocal pe_tail
                ptr, tgt = new_mm_tile("ptr", width=T, dtype=BF16,
                                       parts=RES_DIM, touch=False)
                for nn_ in range(8):
                    t_ = nc.tensor.transpose(
                        ptr[:, nn_ * 128:(nn_ + 1) * 128],
                        rb[:, nn_ * RES_DIM:(nn_ + 1) * RES_DIM],
                        ident,
                    )
                    _order(t_, pe_tail)
                    pe_tail = t_
                rT = sbrt.tile([RES_DIM, T], BF16, tag="rT", bufs=3, name="rT")
                dve(nc.vector.tensor_copy(rT[:], ptr[:]))
                tag_state[tgt] = (rT, 0, 0)
                return rT

            rb_next = emit_conv(0)
            rt_next = emit_tr(0, rb_next)
            h3a_prev = [None]
            pending_l4 = []
            for i in range(NCH):
                rT = rt_next

                # one ACT touch absorbs every SiLU output-buffer WAW of this
                # chunk (reads last chunk's h3a; its sem retired during the
                # h3b silu)
                if h3a_prev[0] is not None:
                    act_touch(h3a_prev[0][0:1, 0:1])

                # L1: 6 chambers, one [128,1024] psum tile each
                h1s = []
                for cp3 in range(3):
                    ha = sbh.tile([128, T], BF16, tag="h1", bufs=6, name="h1a")
                    hb = sbh.tile([128, T], BF16, tag="h1", bufs=6, name="h1b")
                    pa, ta = new_mm_tile("pm1a")
                    ca, cb = 2 * cp3, 2 * cp3 + 1
                    for s in range(2):
                        mm(pa[:, s * 512:(s + 1) * 512],
                           wi[0:RES_DIM, W1C + ca * 128:W1C + (ca + 1) * 128],
                           rT[:, s * 512:(s + 1) * 512], start=True, stop=True)
                    pb, tb = new_mm_tile("pm1b")
                    for s in range(2):
                        mm(pb[:, s * 512:(s + 1) * 512],
                           wi[0:RES_DIM, W1C + cb * 128:W1C + (cb + 1) * 128],
                           rT[:, s * 512:(s + 1) * 512], start=True, stop=True)
                    silu(ha[:], pa[:], wf[:, B1C + ca:B1C + ca + 1],
                         ha, ta, 0, 0)
                    silu(hb[:], pb[:], wf[:, B1C + cb:B1C + cb + 1],
                         hb, tb, 0, 0)
                    h1s.extend([ha, hb])
                    if cp3 == 0 and pending_l4:
                        # last chunk's L4 rides in PE slack during L1 silus
                        pending_l4.pop(0)()

                # next chunk's res conversion can start as soon as DVE is free
                if i + 1 < NCH:
                    rb_next = emit_conv(i + 1)

                # L2: 3 pairs, both chambers stacked on out partitions
                if i == 0:
                    # one-time observe of the wa DMA lane (pm4 cells are safe
                    # scratch until L4(0) opens the accumulation group)
                    pe_touch(wa[0:1, 0:2], pm4[0:1, 2:4])
                l2t = []
                for pr in range(3):
                    pm2, tg2 = new_mm_tile("pm2")
                    for s in range(2):
                        mm(pm2[:, s * 512:(s + 1) * 512],
                           wa[:, W2BC + pr * 128:W2BC + (pr + 1) * 128],
                           h1s[2 * pr + 1][:, s * 512:(s + 1) * 512],
                           start=True, stop=False)
                        mm(pm2[0:64, s * 512:(s + 1) * 512],
                           wa[:, W2C + 2 * pr * 64:W2C + (2 * pr + 1) * 64],
                           h1s[2 * pr][:, s * 512:(s + 1) * 512],
                           start=False, stop=True)
                    l2t.append((pm2, tg2))
                h2s = []
                for pr in range(3):
                    pm2, tg2 = l2t[pr]
                    h2 = sbh.tile([128, T], BF16, tag="h2", bufs=4, name="h2")
                    silu(h2[:], pm2[:], wf[:, B2PC + pr:B2PC + pr + 1],
                         h2, tg2, 0, 0)
                    h2s.append(h2)

                # L3: pairs 0,1 merged into one [128,1024] tile; pair-0
                # (ready first, zero-padded to full width) opens each half's
                # psum group, pair-1 accumulates
                pa3, ta3 = new_mm_tile("pm3")
                for s in range(2):
                    mm(pa3[:, s * 512:(s + 1) * 512],
                       wa[:, W3AC:W3AC + 128],
                       h2s[0][:, s * 512:(s + 1) * 512], start=True, stop=False)
                for s in range(2):
                    mm(pa3[:, s * 512:(s + 1) * 512],
                       wa[:, W3BC:W3BC + 128],
                       h2s[1][:, s * 512:(s + 1) * 512], start=False, stop=True)

                # transposes for the next chunk slot in here: their tag WAR
                # (silu pm2_1) was just observed by pa3's matmuls
                if i + 1 < NCH:
                    rt_next = emit_tr(i + 1, rb_next)

                # pair 2 sample-folded into [128,512]
                h3a = sbh.tile([128, T], BF16, tag="h3a", bufs=3, name="h3a")
                h3b = sbh.tile([128, 512], BF16, tag="h3b", bufs=3, name="h3b")
                pc3, tc3 = new_mm_tile("pm3b", width=512)
                mm(pc3[:, 0:512], wa[:, W3C1:W3C1 + 128],
                   h2s[2][:, 0:512], start=True, stop=False)
                mm(pc3[:, 0:512], wa[:, W3C2:W3C2 + 128],
                   h2s[2][:, 512:1024], start=False, stop=True)
                silu(h3a[:], pa3[:], wf[:, B3PC:B3PC + 1], h3a, ta3, 0, 0)
                silu(h3b[:], pc3[:], wf[:, B3P2C:B3P2C + 1], h3b, tc3, 0, 0)
                h3a_prev[0] = h3a

                # L4 deferred into the next chunk's L1: accumulate raw rows
                # 6i:6i+6 into the persistent [96,1024] psum tile
                def emit_l4(i=i, h3a=h3a, h3b=h3b):
                    if i == 0:
                        # observe the W4-stack DMA lane; pm4 cells are safe:
                        # the first real matmul start=True re-zeroes
                        pe_touch(wz[0:1, 0:2], pm4[0:1, 0:2])
                    for s in range(2):
                        mm(pm4[0:96, s * 512:(s + 1) * 512],
                           wz[:, W4AC + 96 * i:W4AC + 96 * (i + 1)],
                           h3a[:, s * 512:(s + 1) * 512],
                           start=(i == 0), stop=False)
                        mm(pm4[0:96, s * 512:(s + 1) * 512],
                           wz[64 * s:64 * s + 64,
                              W4BC + 96 * i:W4BC + 96 * (i + 1)],
                           h3b[64 * s:64 * s + 64, 0:512],
                           start=False, stop=(i == NCH - 1))
                pending_l4.append(emit_l4)

            if pending_l4:
                pending_l4.pop(0)()

            # ---- tail: raw materialization + coupled sigmoid recurrence ----
            # Quarter-pipelined: raw (ACT Identity+bias from PSUM) and the
            # seed sigmoid stream per 256-col quarter, with DVE copying raw
            # to bf16 behind them, so iteration 1 starts as soon as the
            # first quarter is ready. Only ACT reads pm4. Recurrence state
            # is bf16 with ping-pong act buffers; the final iteration writes
            # fp32. Three psum [128,256] tiles are allocated once and reused
            # across all 20 quarter-matmuls (same-tile rewrites need no
            # pool-WAW sem; the sigmoid-read WAR paces the rotation).
            NQ = 4
            QW = T // NQ
            for q in range(NQ):
                lo, hi = q * QW, (q + 1) * QW
                silu(raw_sb[:, lo:hi], pm4[0:96, lo:hi],
                     wf[0:96, B4C:B4C + 1], None, None, 0, 0,
                     func=AF.Identity)
                silu(act_r[:, lo:hi], pm4[0:96, lo:hi],
                     wf[0:96, B4C:B4C + 1], None, None, 0, 0, func=AF.Sigmoid)
                dve(nc.vector.tensor_copy(raw_r[:, lo:hi], raw_sb[:, lo:hi]))
            # output DMAs ride the SWDGE queues (Pool engine) so they carry
            # only their data wait; the 8 HW queues stay with the inputs
            nc.gpsimd.dma_start(out=raw_d[:], in_=raw_sb[:])

            pm5s = [new_mm_tile(f"pm5{t}", width=QW)[0] for t in range(3)]
            bufs = [act_r, act_r2]
            for kk in range(CF_ITERS):
                src = bufs[kk % 2]
                dst = bufs[(kk + 1) % 2] if kk < CF_ITERS - 1 else act_o
                if kk >= 1:
                    # advance ACT's clock past the previous iteration's
                    # sigmoids (absorbs the ping-pong WAW two iters back)
                    act_touch(src[0:1, 0:1])
                for q in range(NQ):
                    lo, hi = q * QW, (q + 1) * QW
                    pm5 = pm5s[(kk * NQ + q) % 3]
                    mm(pm5[0:96, 0:QW],
                       wz[0:96, CDC:CDC + 96],
                       src[:, lo:hi], start=True, stop=False)
                    mm(pm5[0:96, 0:QW],
                       wz[0:96, I96C:I96C + 96],
                       raw_r[:, lo:hi], start=False, stop=True)
                    silu(dst[:, lo:hi], pm5[0:96, 0:QW],
                         0.0, dst, None, 0, lo, func=AF.Sigmoid)
                    if kk == CF_ITERS - 1:
                        nc.gpsimd.dma_start(out=act_d[:, lo:hi],
                                            in_=act_o[:, lo:hi])

    return nc


def _pack_consts(W1, b1, W2, b2, W3, b3, W4, b4, coupling, decay):
    wf = np.zeros((128, FCOLS), dtype=np.float32)
    for c in range(6):
        wf[:, B1C + c] = b1[c]
    for pr in range(3):
        wf[0:64, B2PC + pr] = b2[2 * pr]
        wf[64:128, B2PC + pr] = b2[2 * pr + 1]
    for c in range(4):
        wf[c * 32:(c + 1) * 32, B3PC] = b3[c]
    for s in range(2):
        wf[64 * s:64 * s + 32, B3P2C] = b3[4]
        wf[64 * s + 32:64 * s + 64, B3P2C] = b3[5]
    wf[0:96, B4C] = np.tile(b4, 16)

    wi = np.zeros((128, ICOLS), dtype=np.float32)
    wi[:, IDC:IDC + 128] = np.eye(128, dtype=np.float32)
    for c in range(6):
        wi[0:RES_DIM, W1C + c * 128:W1C + (c + 1) * 128] = W1[c]

    wa = np.zeros((128, ACOLS), dtype=np.float32)
    for c in range(6):
        wa[0:128, W2C + c * 64:W2C + (c + 1) * 64] = W2[c]
    for pr in range(3):
        # odd chamber shifted to out rows 64:127; cols 0:64 stay zero so
        # start=True clears the even chamber's rows for the accumulate
        wa[:, W2BC + pr * 128 + 64:W2BC + (pr + 1) * 128] = W2[2 * pr + 1]
    wa[0:64, W3AC:W3AC + 32] = W3[0]
    wa[64:128, W3AC + 32:W3AC + 64] = W3[1]
    wa[0:64, W3BC + 64:W3BC + 96] = W3[2]
    wa[64:128, W3BC + 96:W3BC + 128] = W3[3]
    wa[0:64, W3C1:W3C1 + 32] = W3[4]
    wa[64:128, W3C1 + 32:W3C1 + 64] = W3[5]
    wa[0:64, W3C2 + 64:W3C2 + 96] = W3[4]
    wa[64:128, W3C2 + 96:W3C2 + 128] = W3[5]

    wz = np.zeros((128, ZCOLS), dtype=np.float32)
    for i in range(16):
        ba = W4AC + 96 * i
        for c in range(4):
            wz[c * 32:(c + 1) * 32, ba + 6 * i + c] = W4[c]
        bb = W4BC + 96 * i
        for s in range(2):
            wz[64 * s:64 * s + 32, bb + 6 * i + 4] = W4[4]
            wz[64 * s + 32:64 * s + 64, bb + 6 * i + 5] = W4[5]
    cd = (decay[:, None] * coupling * CF_K).astype(np.float32)
    for g in range(16):
        wz[6 * g:6 * g + 6, CDC + 6 * g:CDC + 6 * g + 6] = cd
    wz[0:96, I96C:I96C + 96] = np.eye(96, dtype=np.float32)
    return (wf, wi.astype(ml_dtypes.bfloat16), wa.astype(ml_dtypes.bfloat16),
            wz.astype(ml_dtypes.bfloat16))


def _unshard(per_core, key):
    """[96, T] group layout -> [BS, 6] per core, concat to [B, 6].

    Chunk 0: sample p*8+n8. Chunks 1-3: 1024 + p*24 + (i-1)*8 + n8.
    Chunks 4-15: 4096 + p*96 + (i-4)*8 + n8."""
    outs = []
    for r in per_core:
        a = r[key].reshape(NCH, 6, 8, 128)             # [i, c, n8, p]
        out = np.empty((BS, 6), dtype=a.dtype)
        out[0:T] = a[0].transpose(2, 1, 0).reshape(T, 6)
        out[T:4 * T] = a[1:4].transpose(3, 0, 2, 1).reshape(3 * T, 6)
        out[4 * T:] = a[4:].transpose(3, 0, 2, 1).reshape(12 * T, 6)
        outs.append(out)
    return np.concatenate(outs, axis=0)


def kernel(res, W1, b1, W2, b2, W3, b3, W4, b4, coupling, decay):
    res = np.asarray(res, dtype=np.float32)
    args = [np.asarray(a, dtype=np.float32)
            for a in (W1, b1, W2, b2, W3, b3, W4, b4, coupling, decay)]
    wf, wi, wa, wz = _pack_consts(*args)

    nc = build_module()
    in_maps = [
        {"res": np.ascontiguousarray(res[i * BS:(i + 1) * BS]),
         "wf": wf, "wi": wi, "wa": wa, "wz": wz}
        for i in range(NCORES)
    ]
    results = run_bass_kernel_spmd(nc, in_maps, core_ids=list(range(NCORES)))
    act = _unshard(results.results, "act_out")
    raw = _unshard(results.results, "raw_out")
    return act, raw
